# revision 12
# baseline (speedup 1.0000x reference)
"""Equivariant 3x3x3 conv (e3nn-style) on 8 Trainium2 NeuronCores.

Strategy:
  - The whole module reduces to a dense 3D conv with a 3x3x3x64x64 kernel:
    the position-dependent equivariant kernel is built on the host (tiny
    einsum over radial basis + CG blocks), and the pointwise self-connection
    is folded into the center tap.
  - Sharding: data-parallel over batch (4) x spatial X halves (2) = 8 cores.
    Halo of 1 voxel in X is materialized host-side (overlapping shards).
  - Per core: the padded volume [18,34,34] is flattened so that every conv
    tap is a constant offset into one contiguous buffer. The conv is then
    27 shifted matmuls (64x64 kernel blocks) per 512-voxel output window,
    accumulated in PSUM. Taps are packed two-at-a-time into K=128 matmuls
    using a duplicated, z-shifted copy of the volume on SBUF partitions
    64..127. Compute dtype float32r (full-rate fp32 path on TRN2 PE).
"""

import math
import numpy as np

import concourse.bacc as bacc
import concourse.bass as bass
import concourse.mybir as mybir
import concourse.tile as tile
from concourse.bass_utils import run_bass_kernel_spmd

# ---- problem geometry (hardcoded per spec) ----
MUL = 16
NRB = 8
DIM = 64
B, G = 4, 32

XS = 18            # 16 x-slabs + 2 halo
PY, PZ = 34, 34    # 32 + 2 pad
L = XS * PY * PZ   # 20808 padded voxels
NT = 164           # number of 128-voxel tiles (164*128 = 20992 >= L)
LP = NT * 128      # 20992
Q0 = 1 * (PY * PZ) + 1 * PZ + 1   # 1191, first valid output voxel
NWP = 18           # window pairs of 1024 output voxels: covers [Q0, Q0+18432)

F32 = mybir.dt.float32
F32R = mybir.dt.float32r

_nc_cache = {}


# ---------------- host-side kernel construction ----------------

def _constants():
    r = 3.0 / 2
    axes = [np.arange(-math.floor(r), math.floor(r) + 1.0) for _ in range(3)]
    lattice = np.stack(np.meshgrid(*axes, indexing="ij"), axis=-1)
    d = np.linalg.norm(lattice, axis=-1)
    values = np.linspace(0.0, r, NRB + 2)[1:-1]
    step = values[1] - values[0]
    diff = (d[..., None] - values) / step
    inside = diff ** 2 < 1.0
    emb = np.where(
        inside,
        1.14136 * np.exp(2.0) * np.exp(-2.0 / np.where(inside, 1.0 - diff ** 2, 1.0)),
        0.0,
    )
    n = lattice / np.maximum(d[..., None], 1e-12)
    sh1 = np.sqrt(3.0) * n
    return emb.astype(np.float32), sh1.astype(np.float32)


_EMB, _SH1 = _constants()


def _build_conv_kernel(W_sc0, W_sc1, w1, w2, w3, w4):
    """Full [3,3,3,64,64] conv kernel with self-connection folded in."""
    emb, sh1 = _EMB, _SH1
    norm = 27.0
    W1, W2, W3, W4 = [
        np.einsum("xyzk,kio->xyzio", emb, w.astype(np.float64)) / norm
        for w in (w1, w2, w3, w4)
    ]
    alpha = 1.0 / np.sqrt(2.0 * MUL)
    sh1 = sh1.astype(np.float64)
    k00 = alpha * W1
    k01 = alpha * np.einsum("xyzio,xyzm->xyziom", W2, sh1).reshape(3, 3, 3, MUL, 3 * MUL)
    k10 = (alpha / np.sqrt(3.0)) * np.einsum(
        "xyzio,xyzm->xyzimo", W4, sh1
    ).reshape(3, 3, 3, 3 * MUL, MUL)
    k11 = alpha * np.einsum(
        "xyzio,mn->xyzimon", W3, np.eye(3)
    ).reshape(3, 3, 3, 3 * MUL, 3 * MUL)
    k = np.concatenate(
        [
            np.concatenate([k00, k01], axis=-1),
            np.concatenate([k10, k11], axis=-1),
        ],
        axis=-2,
    )  # [3,3,3,64,64] (i, o)

    # fold self-connection (pointwise linear) into center tap
    inv = 1.0 / np.sqrt(MUL)
    ksc = np.zeros((DIM, DIM))
    ksc[:MUL, :MUL] = W_sc0.astype(np.float64) * inv
    for c in range(3):
        ksc[MUL + 3 * np.arange(MUL)[:, None] + c,
            MUL + 3 * np.arange(MUL)[None, :] + c] += W_sc1.astype(np.float64) * inv
    k[1, 1, 1] += ksc
    return k.astype(np.float32)


# ---------------- device program ----------------

def _build_program(reps=1, variant="packed"):
    """variant: 'packed' = both 512-voxel windows of a pair col-packed into one
    PSUM bank (partitions 0:64 / 64:128) for PE col-group concurrency.
    'split' = one bank per window, partition base 0 (CoreSim-compatible)."""
    key = (reps, variant)
    if key in _nc_cache:
        return _nc_cache[key]

    nc = bacc.Bacc("TRN2", target_bir_lowering=False, debug=False)
    x_h = nc.dram_tensor("x", (LP, DIM), F32, kind="ExternalInput")
    wp_h = nc.dram_tensor("wp", (128, 9 * 64), F32, kind="ExternalInput")
    wq_h = nc.dram_tensor("wq", (128, 128), F32, kind="ExternalInput")
    ws_h = nc.dram_tensor("ws", (64, 9 * 64), F32, kind="ExternalInput")
    id128_h = nc.dram_tensor("id128", (128, 128), F32, kind="ExternalInput")
    idt_h = nc.dram_tensor("idt", (128, 64), F32, kind="ExternalInput")
    out_h = nc.dram_tensor("out", (LP, DIM), F32, kind="ExternalOutput")

    with tile.TileContext(nc) as tc:
        with (
            tc.tile_pool(name="const", bufs=1) as cpool,
            tc.tile_pool(name="stage", bufs=3) as spool,
            tc.tile_pool(name="pin", bufs=2, space="PSUM") as pin,
            tc.tile_pool(name="pconv", bufs=2, space="PSUM") as pconv,
            tc.tile_pool(name="pout", bufs=2, space="PSUM") as pout,
        ):
            ident = cpool.tile([128, 128], F32)
            identt = cpool.tile([128, 64], F32)
            nc.sync.dma_start(ident[:], id128_h.ap())
            nc.sync.dma_start(identt[:], idt_h.ap())

            wp_f = cpool.tile([128, 9 * 64], F32)
            wq_f = cpool.tile([128, 128], F32)
            ws_f = cpool.tile([64, 9 * 64], F32)
            nc.sync.dma_start(wp_f[:], wp_h.ap())
            nc.sync.dma_start(wq_f[:], wq_h.ap())
            nc.sync.dma_start(ws_f[:], ws_h.ap())
            wp_r = cpool.tile([128, 9 * 64], F32R)
            wq_r = cpool.tile([128, 128], F32R)
            ws_r = cpool.tile([64, 9 * 64], F32R)
            nc.vector.tensor_copy(wp_r[:], wp_f[:])
            nc.vector.tensor_copy(wq_r[:], wq_f[:])
            nc.vector.tensor_copy(ws_r[:], ws_f[:])

            x_raw = cpool.tile([128, NT * 128], F32)
            x_t = cpool.tile([128, LP], F32R)

            for _ in range(reps):
                # ---- input: HBM -> SBUF (voxel-major), 256B/voxel rows ----
                xr3 = x_raw[:].rearrange("p (t w) -> p t w", w=128)
                nc.sync.dma_start(
                    xr3[:, :, 0:64],
                    x_h.ap().rearrange("(t p) c -> p t c", p=128),
                )
                # z+1-shifted duplicate into the B columns (SBUF->SBUF)
                nc.sync.dma_start(
                    x_raw[0:127].rearrange("p (t w) -> p t w", w=128)[:, :, 64:128],
                    x_raw[1:128].rearrange("p (t w) -> p t w", w=128)[:, :, 0:64],
                )
                nc.sync.dma_start(
                    x_raw[127:128].rearrange("p (t w) -> p t w", w=128)[:, 0:163, 64:128],
                    x_raw[0:1].rearrange("p (t w) -> p t w", w=128)[:, 1:164, 0:64],
                )
                # last tile's B-half on partition 127 has no shift source;
                # fill with defined data (never read by the conv matmuls)
                # so the transpose doesn't see uninitialized SBUF.
                nc.sync.dma_start(
                    x_raw[127:128, (NT - 1) * 128 + 64: NT * 128],
                    x_raw[0:1, (NT - 1) * 128: (NT - 1) * 128 + 64],
                )

                # ---- transpose to channel-major x_t (dup halves), f32r round ----
                for blk in range(NT // 4):
                    ps = pin.tile([128, 512], F32)
                    for t4 in range(4):
                        t = blk * 4 + t4
                        nc.tensor.matmul(
                            ps[:, t4 * 128:(t4 + 1) * 128],
                            x_raw[:, t * 128:(t + 1) * 128],
                            ident[:],
                            is_transpose=True,
                            start=(t4 == 0),
                            stop=(t4 == 3),
                        )
                    nc.scalar.copy(x_t[:, blk * 512:(blk + 1) * 512], ps[:])

                # ---- conv: 18 window pairs of 2x512 output voxels ----
                for wpi in range(NWP):
                    q0 = Q0 + 1024 * wpi
                    off00 = -(PY * PZ) - PZ - 1
                    if variant == "packed":
                        ps = pconv.tile([128, 512], F32)
                        outs = [ps[0:64, :], ps[64:128, :]]
                        # first matmul covers the full bank (M=128: pair-j0
                        # weights for the W1 half, zeros for the W2 half) so
                        # one start brackets both halves.
                        nc.tensor.matmul(
                            ps[:, :], wq_r[:], x_t[:, q0 + off00: q0 + off00 + 512],
                            start=True, stop=False,
                        )
                        nc.tensor.matmul(
                            ps[64:128, :], wp_r[:, 0:64],
                            x_t[:, q0 + 512 + off00: q0 + 512 + off00 + 512],
                            start=False, stop=False,
                        )
                    else:
                        psa = pconv.tile([64, 512], F32, tag="psa")
                        psb = pconv.tile([64, 512], F32, tag="psb")
                        outs = [psa[:], psb[:]]
                        for w in range(2):
                            nc.tensor.matmul(
                                outs[w], wp_r[:, 0:64],
                                x_t[:, q0 + w * 512 + off00: q0 + w * 512 + off00 + 512],
                                start=True, stop=False,
                            )
                    for j in range(1, 9):
                        dx, dy = divmod(j, 3)
                        off = (dx - 1) * (PY * PZ) + (dy - 1) * PZ - 1  # dz=0 tap
                        lw = wp_r[:, j * 64:(j + 1) * 64]
                        for w in range(2):
                            nc.tensor.matmul(
                                outs[w],
                                lw,
                                x_t[:, q0 + w * 512 + off: q0 + w * 512 + off + 512],
                                start=False, stop=False,
                            )
                    for j in range(9):
                        dx, dy = divmod(j, 3)
                        off = (dx - 1) * (PY * PZ) + (dy - 1) * PZ + 1  # dz=2 tap
                        lw = ws_r[:, j * 64:(j + 1) * 64]
                        for w in range(2):
                            nc.tensor.matmul(
                                outs[w],
                                lw,
                                x_t[0:64, q0 + w * 512 + off: q0 + w * 512 + off + 512],
                                start=False,
                                stop=(j == 8),
                            )

                    st = spool.tile([128, 512], F32, tag="st")
                    if variant == "packed":
                        nc.vector.tensor_copy(st[:], ps[:])
                    else:
                        nc.vector.tensor_copy(st[0:64, :], psa[:])
                        nc.vector.tensor_copy(st[64:128, :], psb[:])

                    # transpose back to voxel-major
                    po = pout.tile([128, 512], F32)
                    for k in range(8):
                        w, b4 = divmod(k, 4)
                        nc.tensor.matmul(
                            po[:, k * 64:(k + 1) * 64],
                            st[w * 64:(w + 1) * 64, b4 * 128:(b4 + 1) * 128],
                            identt[w * 64:(w + 1) * 64, :],
                            is_transpose=True,
                            start=(k == 0),
                            stop=(k == 7),
                        )
                    ob = spool.tile([128, 512], F32, tag="ob")
                    nc.vector.tensor_copy(ob[:], po[:])
                    nc.sync.dma_start(
                        out_h.ap()[q0: q0 + 1024, :].rearrange("(k p) c -> p k c", p=128),
                        ob[:].rearrange("p (k c) -> p k c", c=64),
                    )

    nc.compile()
    _nc_cache[key] = nc
    return nc


# ---------------- host-side shard/gather + entry point ----------------

def _make_in_maps(x, W_sc0, W_sc1, w1, w2, w3, w4):
    k = _build_conv_kernel(W_sc0, W_sc1, w1, w2, w3, w4)

    wp = np.zeros((128, 9 * 64), np.float32)
    ws = np.zeros((64, 9 * 64), np.float32)
    for j in range(9):
        dx, dy = divmod(j, 3)
        wp[0:64, j * 64:(j + 1) * 64] = k[dx, dy, 0]
        wp[64:128, j * 64:(j + 1) * 64] = k[dx, dy, 1]
        ws[:, j * 64:(j + 1) * 64] = k[dx, dy, 2]
    wq = np.zeros((128, 128), np.float32)
    wq[:, 0:64] = wp[:, 0:64]

    id128 = np.eye(128, dtype=np.float32)
    idt = np.concatenate([np.eye(64), np.eye(64)], axis=0).astype(np.float32)

    in_maps = []
    for core in range(8):
        b, xh = divmod(core, 2)
        xp = np.zeros((LP, DIM), np.float32)
        vol = xp[:L].reshape(XS, PY, PZ, DIM)
        gx0 = xh * 16 - 1
        s0 = max(0, -gx0)
        g0 = gx0 + s0
        g1 = min(G, gx0 + XS)
        s1 = s0 + (g1 - g0)
        vol[s0:s1, 1:33, 1:33, :] = x[b, g0:g1]
        in_maps.append({
            "x": xp,
            "wp": wp,
            "wq": wq,
            "ws": ws,
            "id128": id128,
            "idt": idt,
        })
    return in_maps


def kernel(x, W_sc0, W_sc1, w1, w2, w3, w4, _reps=1, _return_raw=False,
           _variant="packed"):
    x = np.asarray(x, dtype=np.float32)
    in_maps = _make_in_maps(
        x,
        np.asarray(W_sc0, np.float32), np.asarray(W_sc1, np.float32),
        np.asarray(w1, np.float32), np.asarray(w2, np.float32),
        np.asarray(w3, np.float32), np.asarray(w4, np.float32),
    )
    nc = _build_program(reps=_reps, variant=_variant)
    res = run_bass_kernel_spmd(nc, in_maps, core_ids=list(range(8)))
    if _return_raw:
        return res
    out = np.empty((B, G, G, G, DIM), np.float32)
    for core in range(8):
        b, xh = divmod(core, 2)
        vol = res.results[core]["out"][:L].reshape(XS, PY, PZ, DIM)
        out[b, xh * 16:(xh + 1) * 16] = vol[1:17, 1:33, 1:33, :]
    return out
